# revision 1
# baseline (speedup 1.0000x reference)
"""Trainium2 Bass kernel for nn_CapsuleLinear (k-means 'dot' routing, 3 iters).

Math (per example b):
  priors[o,i,v] = sum_l W[o,i,v,l] * x[b,i,l]
  out0 = mean_i priors
  3x: n = normalize(out); logits[o,i] = sum_v priors*n; probs = softmax_o(logits);
      out[o,v] = sum_i probs*priors
  result = squash(out) + bias

Sharding: data-parallel over batch B=64 across 8 cores (8 examples/core).

Per-core layout (P = 128 partitions = (i_p in 0..15, b in 0..7), p = i_p*8+b):
  priors SBUF fp16 [128, ib=32, v=16, o=64], full i = ib*16 + i_p.
  Priors are produced by PE matmuls: for each ib, lhsT = block-diagonal x
  ([128=(i_sub,l), 128=(i_sub,b)]) and rhs = W2[ib] ([128=(i_sub,l), 1024=(o,v)]),
  giving PSUM [(i_sub,b), (o,v)]; ACT copies each block to SBUF as fp16.
  out0 = sum_i priors is accumulated in parallel (even ibs: PE ones-matmuls
  over the priors tiles into PSUM; odd ibs: fp32 adds on the DVE, folded in
  by a final ones-matmul). The fixed 0/1 "ones" matrix (1 where p%8 == m%8)
  reduces the partition dim AND re-broadcasts the result over all i_p rows,
  so the routing state never needs a partition broadcast.
  Routing iterations: DVE does the big fp16 2x elementwise muls and the
  v-reduction tree; ACT does exp (and the PSUM squares/copies); the PE does
  the entire i-reduction (out = sum_i probs*priors) as 64 PSUM-accumulating
  ones-matmuls per iteration, interleaved with the DVE mul chunks.
  Measured on trn2 (8 cores, core 0 traced): ~272 us; absmax rel err ~1.2e-3
  (fp16 rounding dominates the error).
"""

import os

import numpy as np

import concourse.bacc as bacc
import concourse.tile as tile
from concourse import mybir
from concourse.bass_utils import run_bass_kernel_spmd

B, I, O, V, L = 64, 512, 64, 16, 8
NCORES = 8
BL = B // NCORES  # 8 examples per core
IB = I // 16  # 32 blocks of 16 i's
IP = 16  # i_p values per partition group

f32 = mybir.dt.float32
f16 = mybir.dt.float16

LAST_RESULT = None  # stash of BassKernelResults for test harness


def _build_kernel():
    nc = bacc.Bacc(
        "TRN2",
        target_bir_lowering=False,
        debug=False,
        enable_asserts=False,
        num_devices=NCORES,
    )
    w2_d = nc.dram_tensor("w2", [IB, 128, O * V], f16, kind="ExternalInput")
    xdg_d = nc.dram_tensor("xdg", [IB, 128, 128], f16, kind="ExternalInput")
    ones_d = nc.dram_tensor("onesd", [128, 128], f16, kind="ExternalInput")
    bias_d = nc.dram_tensor("biasT", [V, O], f32, kind="ExternalInput")
    out_d = nc.dram_tensor("out", [BL, V, O], f32, kind="ExternalOutput")

    with tile.TileContext(nc) as tc:
        _body(nc, tc, w2_d, xdg_d, ones_d, bias_d, out_d)
    nc.compile()
    return nc


def _body(nc, tc, w2_d, xdg_d, ones_d, bias_d, out_d):
    AL = mybir.AluOpType
    X = mybir.AxisListType.X
    AF = mybir.ActivationFunctionType

    from contextlib import ExitStack

    with ExitStack() as ctx:
        big = ctx.enter_context(tc.tile_pool(name="big", bufs=1))
        wp = ctx.enter_context(tc.tile_pool(name="wp", bufs=6))
        xp = ctx.enter_context(tc.tile_pool(name="xp", bufs=4))
        sm = ctx.enter_context(tc.tile_pool(name="sm", bufs=1))
        pr_ps = ctx.enter_context(tc.tile_pool(name="prps", bufs=3, space="PSUM"))
        out_ps = ctx.enter_context(tc.tile_pool(name="outps", bufs=2, space="PSUM"))

        # ---- persistent tiles ----
        priors = big.tile([128, IB, V, O], f16)
        prod = big.tile([128, IB, V, O], f16)
        logits = big.tile([128, IB, O], f32)
        probs = big.tile([128, IB, O], f16)
        ones_t = big.tile([128, 128], f16)
        bias_t = big.tile([BL, V, O], f32)

        onesf = big.tile([128, 128], f32)
        nc.sync.dma_start(out=ones_t[:], in_=ones_d[:])
        nc.vector.tensor_copy(out=onesf[:], in_=ones_t[:])
        nc.sync.dma_start(
            out=bias_t[:], in_=bias_d[:].unsqueeze(0).broadcast_to([BL, V, O])
        )

        # ---- phase 1: priors + out0 ----
        # out0 (in (v,o) order, replicated over i_p) = sum_i priors, built two
        # ways in parallel: even ibs via PE ones-matmuls over the fp16 priors
        # tiles (accumulating in PSUM), odd ibs summed on the DVE in fp32 and
        # folded in with a final ones-matmul pair.
        out0 = out_ps.tile([128, V, O], f32, bufs=1)
        out0f = out0[:].rearrange("p v o -> p (v o)")
        acc = sm.tile([128, O * V], f32, tag="acc")
        accv = acc[:].rearrange("p (v o) -> p v o", v=V)
        for ib in range(IB):
            w = wp.tile([128, O * V], f16, tag="w")
            nc.sync.dma_start(out=w[:], in_=w2_d[ib])
            xd = xp.tile([128, 128], f16, tag="xd")
            nc.sync.dma_start(out=xd[:], in_=xdg_d[ib])
            pp = pr_ps.tile([128, O * V], f32, tag="pp")
            for h in range(2):
                sl = slice(h * 512, (h + 1) * 512)
                nc.tensor.matmul(pp[:, sl], xd[:], w[:, sl], start=True, stop=True)
            # PSUM (o,v) -> SBUF priors[:, ib] in (v, o) order, cast to fp16
            # (on ACT so the DVE stays free)
            ppv = pp[:].rearrange("p (o v) -> p v o", o=O)
            nc.scalar.copy(out=priors[:, ib], in_=ppv)
            if ib % 2 == 0:
                pslc = priors[:, ib].rearrange("p v o -> p (v o)")
                for h in range(2):
                    sl = slice(h * 512, (h + 1) * 512)
                    nc.tensor.matmul(
                        out0f[:, sl],
                        ones_t[:],
                        pslc[:, sl],
                        start=(ib == 0),
                        stop=False,
                        skip_group_check=True,
                    )
            elif ib == 1:
                nc.vector.tensor_scalar_add(accv, ppv, 0.0)
            else:
                nc.vector.tensor_add(accv, accv, ppv)

        # fold the odd-ib fp32 sum into out0 (also re-broadcasts over i_p)
        for h in range(2):
            sl = slice(h * 512, (h + 1) * 512)
            nc.tensor.matmul(
                out0f[:, sl],
                onesf[:],
                acc[:, sl],
                start=False,
                stop=True,
                skip_group_check=True,
            )

        # ---- phase 2: routing iterations ----
        out_prev = out0
        prev_vo = True  # all out states are (v, o) now
        for t in range(3):
            if prev_vo:
                src_ov = out_prev[:].transpose([0, 2, 1])  # [128, O, V] view
                src_vo = out_prev[:]
            else:
                src_ov = out_prev[:]
                src_vo = out_prev[:].transpose([0, 2, 1])
            # n = out / max(||out||, eps)   (scale of out doesn't matter)
            sq = sm.tile([128, O, V], f32, tag="sq")
            nc.scalar.square(sq[:], src_ov)
            nsq = sm.tile([128, O], f32, tag="nsq")
            nc.vector.tensor_reduce(out=nsq[:], in_=sq[:], axis=X, op=AL.add)
            norm = sm.tile([128, O], f32, tag="norm")
            nc.scalar.sqrt(norm[:], nsq[:])
            rn = sm.tile([128, O], f32, tag="rn")
            nc.vector.reciprocal(rn[:], norm[:])
            ntile = sm.tile([128, V, O], f16, tag="ntile")
            nc.vector.tensor_mul(
                ntile[:], src_vo, rn[:].unsqueeze(1).broadcast_to([128, V, O])
            )

            # logits[p, ib, o] = sum_v priors * n
            for c in range(8):
                s = slice(c * 4, (c + 1) * 4)
                nc.vector.tensor_mul(
                    prod[:, s],
                    priors[:, s],
                    ntile[:].unsqueeze(1).broadcast_to([128, 4, V, O]),
                )
            nc.vector.tensor_add(prod[:, :, 0:8], prod[:, :, 0:8], prod[:, :, 8:16])
            nc.vector.tensor_add(prod[:, :, 0:4], prod[:, :, 0:4], prod[:, :, 4:8])
            nc.vector.tensor_add(prod[:, :, 0:2], prod[:, :, 0:2], prod[:, :, 2:4])
            # probs = softmax over o (no max subtraction; |logits| <~ 4).
            # Split by ib-halves so ACT exp overlaps DVE tree/reduce work.
            elog = sm.tile([128, IB, O], f32, tag="elog")
            zs = sm.tile([128, IB], f32, tag="zs")
            rz = sm.tile([128, IB], f32, tag="rz")
            for hh in range(2):
                si = slice(hh * 16, (hh + 1) * 16)
                nc.vector.tensor_add(
                    logits[:, si], prod[:, si, 0], prod[:, si, 1]
                )
                nc.scalar.activation(
                    out=elog[:, si], in_=logits[:, si], func=AF.Exp
                )
                nc.vector.tensor_reduce(
                    out=zs[:, si], in_=elog[:, si], axis=X, op=AL.add
                )
            nc.vector.reciprocal(rz[:], zs[:])
            nc.vector.tensor_mul(
                probs[:], elog[:], rz[:].unsqueeze(2).broadcast_to([128, IB, O])
            )

            # out_new[p, v, o] = sum_i probs * priors
            for c in range(8):
                s = slice(c * 4, (c + 1) * 4)
                nc.vector.tensor_mul(
                    prod[:, s],
                    priors[:, s],
                    probs[:, s].unsqueeze(2).broadcast_to([128, 4, V, O]),
                )
            # i-reduction on the PE: accumulate sum over (i_p, ib) of prod2
            # into PSUM via the block-diag ones matrix (also re-broadcasts
            # the result over all partitions).
            out_new = pr_ps.tile([128, V, O], f32, tag="pp")
            onf = out_new[:].rearrange("p v o -> p (v o)")
            for ib in range(IB):
                pslc = prod[:, ib].rearrange("p v o -> p (v o)")
                for h in range(2):
                    sl = slice(h * 512, (h + 1) * 512)
                    nc.tensor.matmul(
                        onf[:, sl],
                        ones_t[:],
                        pslc[:, sl],
                        start=(ib == 0),
                        stop=(ib == IB - 1),
                        skip_group_check=True,
                    )
            out_prev = out_new
            prev_vo = True

        # ---- squash + bias on partitions 0..7 (b rows) ----
        sq2 = sm.tile([128, O, V], f32, tag="sq")
        src_ov = out_prev[:].transpose([0, 2, 1])
        nc.scalar.square(sq2[:], src_ov)
        nsq2 = sm.tile([128, O], f32, tag="nsq")
        nc.vector.tensor_reduce(out=nsq2[:], in_=sq2[:], axis=X, op=AL.add)
        norm2 = sm.tile([128, O], f32, tag="norm")
        nc.scalar.sqrt(norm2[:], nsq2[:])
        den = sm.tile([128, O], f32, tag="den")
        nc.vector.tensor_scalar_add(den[:], nsq2[:], 1.0)
        rden = sm.tile([128, O], f32, tag="rden")
        nc.vector.reciprocal(rden[:], den[:])
        scl = sm.tile([128, O], f32, tag="scl")
        nc.vector.tensor_mul(scl[:], norm2[:], rden[:])

        outf = sm.tile([BL, V, O], f32, tag="outf")
        nc.vector.tensor_mul(
            outf[:],
            out_prev[0:BL],
            scl[0:BL].unsqueeze(1).broadcast_to([BL, V, O]),
        )
        nc.vector.tensor_add(outf[:], outf[:], bias_t[:])
        nc.sync.dma_start(out=out_d[:], in_=outf[:])


_NC_CACHE = []


def _get_nc():
    if not _NC_CACHE:
        _NC_CACHE.append(_build_kernel())
    return _NC_CACHE[0]


def kernel(x, weight, bias):
    global LAST_RESULT
    x = np.asarray(x, dtype=np.float32)
    weight = np.asarray(weight, dtype=np.float32)
    bias = np.asarray(bias, dtype=np.float32)

    # W2[ib, (i_sub, l), (o, v)] = W[o, ib*16+i_sub, v, l]  (fp16: same byte
    # cost as bf16 but 4x finer mantissa; values are well within fp16 range)
    w2 = (
        np.ascontiguousarray(weight.transpose(1, 3, 0, 2))
        .reshape(IB, 128, O * V)
        .astype(np.float16)
    )
    biasT = np.ascontiguousarray(bias.T)  # [V, O]

    idx = np.arange(128)
    onesd = (idx[:, None] % BL == idx[None, :] % BL).astype(np.float16)

    in_maps = []
    for c in range(NCORES):
        xc = x[c * BL : (c + 1) * BL]  # [BL, I, L]
        xt = np.ascontiguousarray(xc.transpose(1, 2, 0))  # [I, L, BL] = (i, l, b)
        xt4 = xt.reshape(IB, 16, L, BL)
        xdg = np.zeros((IB, 128, 128), dtype=np.float16)
        for s in range(16):
            xdg[:, s * L : (s + 1) * L, s * BL : (s + 1) * BL] = xt4[:, s].astype(
                np.float16
            )
        in_maps.append({"w2": w2, "xdg": xdg, "onesd": onesd, "biasT": biasT})

    nc = _get_nc()
    try:
        res = run_bass_kernel_spmd(nc, in_maps, core_ids=list(range(NCORES)))
    except ModuleNotFoundError:
        # BASS_TRACE was set but this environment lacks the axon NTFF hook
        # module; rerun without tracing.
        os.environ["BASS_NEVER_TRACE"] = "1"
        res = run_bass_kernel_spmd(nc, in_maps, core_ids=list(range(NCORES)))
    LAST_RESULT = res

    outs = []
    for r in res.results:
        o = r["out"]  # [BL, V, O]
        outs.append(np.ascontiguousarray(o.transpose(0, 2, 1)))  # [BL, O, V]
    return np.concatenate(outs, axis=0).astype(np.float32)


if __name__ == "__main__":
    rng = np.random.default_rng(0)
    x = rng.standard_normal((B, I, L), dtype=np.float32)
    w = rng.standard_normal((O, I, V, L), dtype=np.float32) * 0.1
    b = rng.standard_normal((O, V), dtype=np.float32) * 0.1
    out = kernel(x, w, b)
    print("out shape", out.shape, out.dtype)



# revision 5
# speedup vs baseline: 1.3700x; 1.3700x over previous
"""Trainium2 Bass kernel for nn_CapsuleLinear (k-means 'dot' routing, 3 iters).

Math (per example b):
  priors[o,i,v] = sum_l W[o,i,v,l] * x[b,i,l]
  out0 = mean_i priors
  3x: n = normalize(out); logits[o,i] = sum_v priors*n; probs = softmax_o(logits);
      out[o,v] = sum_i probs*priors
  result = squash(out) + bias

Sharding: data-parallel over batch B=64 across 8 cores (8 examples/core).

Per-core layout (P = 128 partitions = (i_p in 0..15, b in 0..7), p = i_p*8+b):
  priors SBUF fp16 [128, ib=32, v=16, o=64], full i = ib*16 + i_p.
  Produced by PE matmuls (block-diag x lhsT vs W2 rhs) into PSUM; the
  PSUM->SBUF fp16 casts are split between ACT (even ib) and DVE (odd ib).
  out0 = sum_i priors: even ibs via PE ones-matmuls (PSUM accumulation),
  odd ibs via a DVE fp16 pairwise tree folded in with a ones-matmul.
  The 0/1 "ones" matrix (1 where p%8 == m%8) reduces the partition dim
  AND re-broadcasts over all i_p rows.
  Routing iterations: DVE does only the two big fp16 muls (priors*n and
  priors*probs, 4 chunks of FD 8192 each) plus the small softmax ops.
  The v-reduction (logits = sum_v prod) runs on the PE as 16 PSUM-
  accumulating identity-matmuls per chunk (strided rhs), pipelined
  behind the DVE mul chunks; exp/zsum are pipelined per 512-col chunk
  on ACT/DVE. The i-reduction (out = sum_i probs*priors) is the PE
  ones-matmul over the prod chunks, also trailing the DVE muls.
"""

import os

import numpy as np

import concourse.bacc as bacc
import concourse.tile as tile
from concourse import mybir
from concourse.bass_utils import run_bass_kernel_spmd

B, I, O, V, L = 64, 512, 64, 16, 8
NCORES = 8
BL = B // NCORES  # 8 examples per core
IB = I // 16  # 32 blocks of 16 i's
NQ = 4  # ib-chunks per pass
QIB = IB // NQ  # 8 ibs per chunk

f32 = mybir.dt.float32
f16 = mybir.dt.float16

LAST_RESULT = None  # stash of BassKernelResults for test harness


def _build_kernel():
    nc = bacc.Bacc(
        "TRN2",
        target_bir_lowering=False,
        debug=False,
        enable_asserts=False,
        num_devices=NCORES,
    )
    w2_d = nc.dram_tensor("w2", [IB, 128, O * V], f16, kind="ExternalInput")
    xdg_d = nc.dram_tensor("xdg", [IB, 128, 128], f16, kind="ExternalInput")
    ones_d = nc.dram_tensor("onesd", [128, 128], f16, kind="ExternalInput")
    iden_d = nc.dram_tensor("idend", [128, 128], f16, kind="ExternalInput")
    bias_d = nc.dram_tensor("biasT", [V, O], f32, kind="ExternalInput")
    out_d = nc.dram_tensor("out", [BL, V, O], f32, kind="ExternalOutput")

    with tile.TileContext(nc) as tc:
        _body(nc, tc, w2_d, xdg_d, ones_d, iden_d, bias_d, out_d)
    nc.compile()
    return nc


def _body(nc, tc, w2_d, xdg_d, ones_d, iden_d, bias_d, out_d):
    AL = mybir.AluOpType
    X = mybir.AxisListType.X
    AF = mybir.ActivationFunctionType

    from contextlib import ExitStack

    with ExitStack() as ctx:
        big = ctx.enter_context(tc.tile_pool(name="big", bufs=1))
        wp = ctx.enter_context(tc.tile_pool(name="wp", bufs=6))
        xp = ctx.enter_context(tc.tile_pool(name="xp", bufs=4))
        sm = ctx.enter_context(tc.tile_pool(name="sm", bufs=1))
        # PSUM: pool A (phase-1 priors pp + per-iter logits halves),
        # pool B (the out states). 4 banks each.
        ps_a = ctx.enter_context(tc.tile_pool(name="psa", bufs=2, space="PSUM"))
        ps_o = ctx.enter_context(tc.tile_pool(name="pso", bufs=2, space="PSUM"))

        # ---- persistent tiles ----
        priors = big.tile([128, IB, V, O], f16)
        prod = big.tile([128, IB, V, O], f16)
        probs = big.tile([128, IB, O], f16)
        elog = big.tile([128, IB, O], f32)
        zs = big.tile([128, IB], f32)
        ones_t = big.tile([128, 128], f16)
        iden_t = big.tile([128, 128], f16)
        bias_t = big.tile([BL, V, O], f32)

        nc.sync.dma_start(out=ones_t[:], in_=ones_d[:])
        nc.sync.dma_start(out=iden_t[:], in_=iden_d[:])
        nc.sync.dma_start(
            out=bias_t[:], in_=bias_d[:].unsqueeze(0).broadcast_to([BL, V, O])
        )

        # ---- phase 1: priors + out0 ----
        # out0 (in (v,o) order, replicated over i_p) = sum_i priors: even ibs
        # via PE ones-matmuls over the fp16 priors tiles (PSUM accumulation),
        # odd ibs via a DVE fp16 pairwise tree, folded in by a ones-matmul.
        out0 = ps_o.tile([128, V, O], f32, tag="out")
        out0f = out0[:].rearrange("p v o -> p (v o)")
        # odd-ib partial sums (fp16 tree): acc1 = sum of 16 odd ibs
        oacc = sm.tile([128, 8, V, O], f16, tag="oacc")
        for ib in range(IB):
            w = wp.tile([128, O * V], f16, tag="w")
            nc.sync.dma_start(out=w[:], in_=w2_d[ib])
            xd = xp.tile([128, 128], f16, tag="xd")
            nc.sync.dma_start(out=xd[:], in_=xdg_d[ib])
            pp = ps_a.tile([128, O * V], f32, tag="pp")
            for h in range(2):
                sl = slice(h * 512, (h + 1) * 512)
                nc.tensor.matmul(pp[:, sl], xd[:], w[:, sl], start=True, stop=True)
            # PSUM (o,v) -> SBUF priors[:, ib] in (v, o) order, cast to fp16.
            # Split between ACT (even) and DVE (odd) so neither is the
            # phase-1 bottleneck.
            ppv = pp[:].rearrange("p (o v) -> p v o", o=O)
            if ib % 2 == 0:
                nc.scalar.copy(out=priors[:, ib], in_=ppv)
                pslc = priors[:, ib].rearrange("p v o -> p (v o)")
                for h in range(2):
                    sl = slice(h * 512, (h + 1) * 512)
                    nc.tensor.matmul(
                        out0f[:, sl],
                        ones_t[:],
                        pslc[:, sl],
                        start=(ib == 0),
                        stop=False,
                        skip_group_check=True,
                    )
            else:
                nc.vector.tensor_copy(out=priors[:, ib], in_=ppv)
        # fp16 pairwise tree over the 16 odd ibs (on the DVE)
        po = priors[:].rearrange("p (hi q) v o -> p hi q v o", q=2)[:, :, 1]
        # po: [128, 16, V, O] = odd ibs. lvl1: 16->8
        nc.vector.tensor_add(oacc[:], po[:, 0:8], po[:, 8:16])
        nc.vector.tensor_add(oacc[:, 0:4], oacc[:, 0:4], oacc[:, 4:8])
        nc.vector.tensor_add(oacc[:, 0:2], oacc[:, 0:2], oacc[:, 2:4])
        nc.vector.tensor_add(oacc[:, 0], oacc[:, 0], oacc[:, 1])
        oaccf = oacc[:, 0].rearrange("p v o -> p (v o)")
        for h in range(2):
            sl = slice(h * 512, (h + 1) * 512)
            nc.tensor.matmul(
                out0f[:, sl],
                ones_t[:],
                oaccf[:, sl],
                start=False,
                stop=True,
                skip_group_check=True,
            )

        # ---- phase 2: routing iterations ----
        out_prev = out0
        for t in range(3):
            src_vo = out_prev[:]
            src_ov = out_prev[:].transpose([0, 2, 1])  # [128, O, V] view
            # n = out * rsqrt(sum_v out^2)  (scale of out doesn't matter)
            sq = sm.tile([128, O, V], f32, tag="sq")
            nc.scalar.square(sq[:], src_ov)
            nsq = sm.tile([128, O], f32, tag="nsq")
            nc.vector.tensor_reduce(out=nsq[:], in_=sq[:], axis=X, op=AL.add)
            norm = sm.tile([128, O], f32, tag="norm")
            nc.scalar.sqrt(norm[:], nsq[:])
            rn = sm.tile([128, O], f32, tag="rn")
            nc.vector.reciprocal(rn[:], norm[:])
            ntile = sm.tile([128, V, O], f16, tag="ntile")
            nc.vector.tensor_mul(
                ntile[:], src_vo, rn[:].unsqueeze(1).broadcast_to([128, V, O])
            )

            # logits[p, ib, o] = sum_v priors * n
            # DVE: prod chunks (FD 8192); PE: 16 identity-matmuls per chunk
            # accumulate the v-reduction into PSUM; ACT/DVE: exp + zsum per
            # 512-col chunk, all pipelined.
            # two logits halves, 2 banks each
            lg0 = ps_a.tile([128, 2, QIB * O], f32, tag="pp", name="lg0")
            lg1 = ps_a.tile([128, 2, QIB * O], f32, tag="pp", name="lg1")
            lg = [lg0, lg1]
            for q in range(NQ):
                s = slice(q * QIB, (q + 1) * QIB)
                nc.vector.tensor_mul(
                    prod[:, s],
                    priors[:, s],
                    ntile[:].unsqueeze(1).broadcast_to([128, QIB, V, O]),
                )
                lgq = lg[q // 2][:, q % 2]  # [128, QIB*O] one bank
                pq = prod[:, s]  # [128, QIB, V, O]
                for v in range(V):
                    nc.tensor.matmul(
                        lgq,
                        iden_t[:],
                        pq[:, :, v],
                        start=(v == 0),
                        stop=(v == V - 1),
                        skip_group_check=True,
                    )
                # softmax pieces for this chunk (no max subtraction;
                # |logits| is small)
                lgq3 = lgq.rearrange("p (q o) -> p q o", o=O)
                nc.scalar.activation(out=elog[:, s], in_=lgq3, func=AF.Exp)
                nc.vector.tensor_reduce(
                    out=zs[:, s], in_=elog[:, s], axis=X, op=AL.add
                )
            rz = sm.tile([128, IB], f32, tag="rz")
            nc.vector.reciprocal(rz[:], zs[:])
            nc.vector.tensor_mul(
                probs[:], elog[:], rz[:].unsqueeze(2).broadcast_to([128, IB, O])
            )

            # out_new[p, v, o] = sum_i probs * priors
            out_new = ps_o.tile([128, V, O], f32, tag="out")
            onf = out_new[:].rearrange("p v o -> p (v o)")
            for q in range(NQ):
                s = slice(q * QIB, (q + 1) * QIB)
                nc.vector.tensor_mul(
                    prod[:, s],
                    priors[:, s],
                    probs[:, s].unsqueeze(2).broadcast_to([128, QIB, V, O]),
                )
                for j in range(QIB):
                    ib = q * QIB + j
                    pslc = prod[:, ib].rearrange("p v o -> p (v o)")
                    for h in range(2):
                        sl = slice(h * 512, (h + 1) * 512)
                        nc.tensor.matmul(
                            onf[:, sl],
                            ones_t[:],
                            pslc[:, sl],
                            start=(ib == 0),
                            stop=(ib == IB - 1),
                            skip_group_check=True,
                        )
            out_prev = out_new

        # ---- squash + bias on partitions 0..7 (b rows) ----
        sq2 = sm.tile([128, O, V], f32, tag="sq")
        src_ov = out_prev[:].transpose([0, 2, 1])
        nc.scalar.square(sq2[:], src_ov)
        nsq2 = sm.tile([128, O], f32, tag="nsq")
        nc.vector.tensor_reduce(out=nsq2[:], in_=sq2[:], axis=X, op=AL.add)
        norm2 = sm.tile([128, O], f32, tag="norm")
        nc.scalar.sqrt(norm2[:], nsq2[:])
        den = sm.tile([128, O], f32, tag="den")
        nc.vector.tensor_scalar_add(den[:], nsq2[:], 1.0)
        rden = sm.tile([128, O], f32, tag="rden")
        nc.vector.reciprocal(rden[:], den[:])
        scl = sm.tile([128, O], f32, tag="scl")
        nc.vector.tensor_mul(scl[:], norm2[:], rden[:])

        outf = sm.tile([BL, V, O], f32, tag="outf")
        nc.vector.tensor_mul(
            outf[:],
            out_prev[0:BL],
            scl[0:BL].unsqueeze(1).broadcast_to([BL, V, O]),
        )
        nc.vector.tensor_add(outf[:], outf[:], bias_t[:])
        nc.sync.dma_start(out=out_d[:], in_=outf[:])


_NC_CACHE = []


def _get_nc():
    if not _NC_CACHE:
        _NC_CACHE.append(_build_kernel())
    return _NC_CACHE[0]


def kernel(x, weight, bias):
    global LAST_RESULT
    x = np.asarray(x, dtype=np.float32)
    weight = np.asarray(weight, dtype=np.float32)
    bias = np.asarray(bias, dtype=np.float32)

    # W2[ib, (i_sub, l), (o, v)] = W[o, ib*16+i_sub, v, l]  (fp16: same byte
    # cost as bf16 but 4x finer mantissa; values are well within fp16 range)
    w2 = (
        np.ascontiguousarray(weight.transpose(1, 3, 0, 2))
        .reshape(IB, 128, O * V)
        .astype(np.float16)
    )
    biasT = np.ascontiguousarray(bias.T)  # [V, O]

    idx = np.arange(128)
    onesd = (idx[:, None] % BL == idx[None, :] % BL).astype(np.float16)
    idend = np.eye(128, dtype=np.float16)

    in_maps = []
    for c in range(NCORES):
        xc = x[c * BL : (c + 1) * BL]  # [BL, I, L]
        xt = np.ascontiguousarray(xc.transpose(1, 2, 0))  # [I, L, BL] = (i, l, b)
        xt4 = xt.reshape(IB, 16, L, BL)
        xdg = np.zeros((IB, 128, 128), dtype=np.float16)
        for s in range(16):
            xdg[:, s * L : (s + 1) * L, s * BL : (s + 1) * BL] = xt4[:, s].astype(
                np.float16
            )
        in_maps.append(
            {"w2": w2, "xdg": xdg, "onesd": onesd, "idend": idend, "biasT": biasT}
        )

    nc = _get_nc()
    try:
        res = run_bass_kernel_spmd(nc, in_maps, core_ids=list(range(NCORES)))
    except ModuleNotFoundError:
        # BASS_TRACE was set but this environment lacks the axon NTFF hook
        # module; rerun without tracing.
        os.environ["BASS_NEVER_TRACE"] = "1"
        res = run_bass_kernel_spmd(nc, in_maps, core_ids=list(range(NCORES)))
    LAST_RESULT = res

    outs = []
    for r in res.results:
        o = r["out"]  # [BL, V, O]
        outs.append(np.ascontiguousarray(o.transpose(0, 2, 1)))  # [BL, O, V]
    return np.concatenate(outs, axis=0).astype(np.float32)


if __name__ == "__main__":
    rng = np.random.default_rng(0)
    x = rng.standard_normal((B, I, L), dtype=np.float32)
    w = rng.standard_normal((O, I, V, L), dtype=np.float32) * 0.1
    b = rng.standard_normal((O, V), dtype=np.float32) * 0.1
    out = kernel(x, w, b)
    print("out shape", out.shape, out.dtype)
